# revision 20
# baseline (speedup 1.0000x reference)
"""Multi-headed attention Trainium2 kernel (v4, bf16, interleaved phases).

Problem: B=4, S=2048, D=1024, H=16, dk=dv=64, fp32 in/out.
  q = einsum("bsd,hdk->bhsk", x, W_Q); k,v similar
  scores = q@k.T/8; attn = softmax(scores); out = attn@v
  y = concat_heads(out) @ W_O

Sharding: 8 cores = 4-way data parallel (batch) x 2-way tensor parallel
(head groups of 8). Core c handles batch c%4, heads 8*(c//4)..+8. Each core
returns a partial y for its batch; host sums the two head-group partials.

All matmul operands bf16 (fp32 PSUM accumulate; absmax rel err ~1e-2 <
2e-2). Everything is SBUF-resident: x^T, Q^T/K^T per head pair, V(+ones
column for the softmax denominator), and all weights. Host passes x
pre-cast to bf16; x^T is produced by XBAR dma_start_transpose (no PE/DVE).

The per-iteration stream is a single software-pipelined sequence ordered
for the in-order PE queue: a short prefix (K and Q(chunk 0) of pair 0),
then per (s-chunk, pair) score/exp/AV groups with the remaining phase-A
projection units (K/Q of later pairs, V per t-tile) injected into the PE
slack of early streams, and W_O of s-chunk sc emitted one chunk late,
spread across the next chunk's pairs (its ot inputs are then long since
ready -- no PE stall, no ACT bubble). ACT (exp of all S^2 scores, the
other near-roofline engine besides PE) starts ~25us into the iteration
instead of after the whole projection phase.
"""

import numpy as np

import concourse.bacc as bacc
import concourse.bass as bass
import concourse.mybir as mybir
import concourse.tile as tile

F32 = mybir.dt.float32
BF16 = mybir.dt.bfloat16
P = 128
DK = 64  # per-head dim; also dv
VW = DK + 1  # 65: v columns + ones column


def build_nc(S, D, HL, num_devices=8, n_iters=1, cfg=None):
    """Build the per-core Bass program. S seq len, D model dim, HL local heads."""
    NSC = S // 512  # s-chunks (phase B)
    NT = S // P  # t-tiles
    ND = D // P  # d-tiles
    NPAIR = HL // 2
    NDC = max(1, D // 512)  # output d chunks
    DC = min(D, 512)
    scale = 1.0 / np.sqrt(np.float64(DK))
    cfg = dict(cfg or {})
    phases = cfg.get("phases", "ab")
    interleave = cfg.get("interleave", 1)
    et_bufs = cfg.get("et_bufs", 5)
    ot_bufs = cfg.get("ot_bufs", 12)
    psBig_bufs = cfg.get("psBig_bufs", 2)
    psAV_bufs = cfg.get("psAV_bufs", 4)
    units_big = cfg.get("units_big", 0)

    nc = bacc.Bacc("TRN2", target_bir_lowering=False, debug=False,
                   num_devices=num_devices)

    # host-preshuffled bf16 inputs, partition-major, 2KB+ contiguous rows:
    # xb: x cast to bf16 (host-side)
    # wq/wk [NPAIR, 128, ND*128]: row p = [w(2p)[dt*128+p, :] | w(2p+1)[dt*128+p, :]]_dt
    # wv [128, ND*HL*64]: row p = [wv[hl][dt*128+p, :]]_(dt, hl)
    # wo [128, NPAIR*D]: row p = [wo[pr*128+p, :]]_pr
    xb = nc.dram_tensor("xb", [S, D], BF16, kind="ExternalInput").ap()
    wq = nc.dram_tensor("wq", [NPAIR, P, ND * P], BF16, kind="ExternalInput").ap()
    wk = nc.dram_tensor("wk", [NPAIR, P, ND * P], BF16, kind="ExternalInput").ap()
    wv = nc.dram_tensor("wv", [P, ND * HL * DK], BF16, kind="ExternalInput").ap()
    wo = nc.dram_tensor("wo", [P, NPAIR * D], BF16, kind="ExternalInput").ap()
    y = nc.dram_tensor("y", [S, D], F32, kind="ExternalOutput").ap()

    from contextlib import ExitStack

    with tile.TileContext(nc) as tc:
        with ExitStack() as ctx:
            pool = lambda name, bufs, **kw: ctx.enter_context(
                tc.tile_pool(name=name, bufs=bufs, **kw)
            )
            persist = pool("persist", 1)
            et_p = pool("exp", et_bufs)
            ot_p = pool("ot", ot_bufs)
            y_p = pool("ysb", 3)
            rl_p = pool("rl", 3)
            rb_p = pool("rb", 3)
            tmp_p = pool("tmp", 2)
            rl0_p = pool("rl0", 3)
            psBig = pool("psBig", psBig_bufs, space="PSUM")  # [128,1024] x2 = 4 banks
            psAV = pool("psAV", psAV_bufs, space="PSUM")     # [128,512] x4 = 4 banks

            # --- persistent tiles: loaded/initialized ONCE (outside For_i) ---
            wqk_sb = []
            for pr in range(NPAIR):
                for w_dram, nm in ((wq, "wq"), (wk, "wk")):
                    wt = persist.tile([P, ND * P], BF16, tag=f"{nm}{pr}", name=f"{nm}{pr}")
                    nc.sync.dma_start(wt[:], w_dram[pr])
                    wqk_sb.append(wt)  # index 2*pr (+1 for wk)

            wv_sb = persist.tile([P, ND * HL * DK], BF16, tag="wv")
            nc.sync.dma_start(wv_sb[:], wv[:])
            wos_all = persist.tile([P, NPAIR * D], BF16, tag="wo")
            nc.sync.dma_start(wos_all[:], wo[:])

            xT = [persist.tile([P, S], BF16, tag=f"xT{dt}", name=f"xT{dt}") for dt in range(ND)]
            kt = [persist.tile([P, S], BF16, tag=f"kt{p}", name=f"kt{p}") for p in range(NPAIR)]
            qt = [persist.tile([P, S], BF16, tag=f"qt{p}", name=f"qt{p}") for p in range(NPAIR)]
            v_ones = persist.tile([P, NT * HL * VW], BF16, tag="vones")
            ones_view = v_ones[:].rearrange(
                "p (t h c) -> p (t h) c", h=HL, c=VW
            )[:, :, DK : DK + 1]
            nc.gpsimd.memset(ones_view, 1.0)

            if n_iters > 1:
                ctx.enter_context(tc.For_i(0, n_iters, 1))

            # --- x^T via XBAR DMA transpose (no PE/DVE involvement) ---
            xq_eng = nc.scalar if cfg.get("xq", "sp") == "act" else nc.sync
            for dt in range(ND):
                xq_eng.dma_start_transpose(
                    xT[dt][:], xb[:, dt * P : (dt + 1) * P]
                )

            # --- phase-A unit emitters (each: one 8-matmul PSUM group + copy,
            # optionally emitted as two 4-matmul halves to smooth PE load) ---
            def emit_qk_half(pr, wi, sh, half, state={}):
                """Project q (wi=0) or k (wi=1) of pair pr for s-chunk sh."""
                dst = (qt if wi == 0 else kt)[pr]
                wt = wqk_sb[2 * pr + wi]
                if half == 0:
                    if units_big:
                        ps = psBig.tile([P, 1024], F32, tag="sc", name=f"qk_{pr}_{wi}_{sh}")[:, :512]
                    else:
                        ps = psBig.tile([P, 512], F32, tag="sc", name=f"qk_{pr}_{wi}_{sh}")
                    state[(pr, wi, sh)] = ps
                    dts = range(0, ND // 2)
                else:
                    ps = state.pop((pr, wi, sh))
                    dts = range(ND // 2, ND)
                for dt in dts:
                    nc.tensor.matmul(
                        ps[:],
                        wt[:, dt * P : (dt + 1) * P],
                        xT[dt][:, sh * 512 : (sh + 1) * 512],
                        start=(dt == 0),
                        stop=(dt == ND - 1),
                    )
                if half == 1:
                    nc.vector.tensor_copy(dst[:, sh * 512 : (sh + 1) * 512], ps[:])

            def emit_qk_unit(pr, wi, sh):
                emit_qk_half(pr, wi, sh, 0)
                emit_qk_half(pr, wi, sh, 1)

            def emit_v_unit(tt):
                if units_big:
                    ps = psBig.tile([P, 1024], F32, tag="sc", name=f"v_{tt}")[:, :512]
                else:
                    ps = psAV.tile([P, 512], F32, tag="ps", name=f"v_{tt}")
                for dt in range(ND):
                    nc.tensor.matmul(
                        ps[:, : HL * DK],
                        xT[dt][:, tt * P : (tt + 1) * P],
                        wv_sb[:, dt * HL * DK : (dt + 1) * HL * DK],
                        start=(dt == 0),
                        stop=(dt == ND - 1),
                    )
                nc.vector.tensor_copy(
                    v_ones[:].rearrange("p (t h c) -> p t h c", h=HL, c=VW)[
                        :, tt, :, :DK
                    ],
                    ps[:, : HL * DK].rearrange("p (h k) -> p h k", h=HL),
                )

            def emit_wo_unit(sc, ots, i):
                dc, st = i // 4, i % 4
                if units_big:
                    psy = psBig.tile([P, 1024], F32, tag="sc", name=f"psy_{sc}_{i}")[:, :512]
                else:
                    psy = psAV.tile([P, 512], F32, tag="ps", name=f"psy_{sc}_{i}")
                for p in range(NPAIR):
                    nc.tensor.matmul(
                        psy[:, :DC],
                        ots[p][:, st * P : (st + 1) * P],
                        wos_all[:, p * D + dc * DC : p * D + (dc + 1) * DC],
                        start=(p == 0),
                        stop=(p == NPAIR - 1),
                    )
                ys = y_p.tile([P, DC], F32, tag="ysb", name=f"ys_{sc}_{i}")
                nc.vector.tensor_copy(ys[:], psy[:, :DC])
                nc.sync.dma_start(
                    y[(sc * 4 + st) * P : (sc * 4 + st + 1) * P,
                      dc * DC : (dc + 1) * DC],
                    ys[:],
                )

            # --- build the phase-A work schedule ---
            if interleave and phases == "ab":
                # prefix: K(p0) fully + Q(p0, sh0); everything else is
                # injected into per-(sc, p) streams as 4-matmul halves,
                # one per g-slot (smoother PE load than full 8-MM units).
                for sh in range(NSC):
                    emit_qk_unit(0, 1, sh)
                emit_qk_unit(0, 0, 0)

                split = cfg.get("split_fillers", 0)

                def halves(units):
                    out = []
                    for pr, wi, sh in units:
                        if split:
                            out.append(lambda pr=pr, wi=wi, sh=sh: emit_qk_half(pr, wi, sh, 0))
                            out.append(lambda pr=pr, wi=wi, sh=sh: emit_qk_half(pr, wi, sh, 1))
                        else:
                            out.append(lambda pr=pr, wi=wi, sh=sh: emit_qk_unit(pr, wi, sh))
                    return out

                fillers = {}  # (sc, p) -> list of thunks
                for pr in (1, 2, 3)[: NPAIR - 1]:
                    units = [(pr, 1, sh) for sh in range(NSC)] + [(pr, 0, 0)]
                    fillers[(0, pr - 1)] = halves(units)
                # Q(p, sh) for sh>=1: inject during s-chunk sh-1
                for sh in range(1, NSC):
                    if sh == 1:
                        # all four Q(p,1) units go into (sc0, p3)
                        fillers[(0, NPAIR - 1)] = halves(
                            [(pr, 0, 1) for pr in range(NPAIR)]
                        )
                    else:
                        for pr in range(NPAIR):
                            fillers.setdefault((sh - 1, pr), []).extend(
                                halves([(pr, 0, sh)])
                            )
            else:
                # serial phase A (ablation)
                for pr in range(NPAIR):
                    for wi in (0, 1):
                        for sh in range(NSC):
                            emit_qk_unit(pr, wi, sh)
                for tt in range(NT):
                    emit_v_unit(tt)
                fillers = {}

            # --- phase B: flat slot stream with AV lagging one group ---
            # Per slot: scores(g) [+injected phase-A unit], exp(g) on ACT,
            # then AV(g-1). The lag keeps every PE instruction ahead of the
            # exp it feeds: AV(j) waits on exp(j), so putting it AFTER
            # scores/exp of slot j+1 means the in-order PE queue never
            # stalls the ACT feed chain.
            def emit_scores(sc, p, g, pse):
                for h in range(2):  # row-packed head pair
                    nc.tensor.matmul(
                        pse[:, h * 512 : (h + 1) * 512],
                        kt[p][h * DK : (h + 1) * DK, g * P : (g + 1) * P],
                        qt[p][h * DK : (h + 1) * DK, sc * 512 : (sc + 1) * 512],
                        start=True,
                        stop=True,
                    )

            def make_av(sc, p, g, et, po_pair):
                def av():
                    for h, po in ((0, po_pair[0]), (1, po_pair[1])):
                        nc.tensor.matmul(
                            po[:VW, :],
                            v_ones[
                                :,
                                (g * HL + 2 * p + h) * VW : (g * HL + 2 * p + h + 1) * VW,
                            ],
                            et[:, h * 512 : (h + 1) * 512],
                            start=(g == 0),
                            stop=(g == NT - 1),
                        )
                return av

            def emit_normalize(sc, p, po_pair, ots):
                # normalize: rows 0:64 divided by row 64 (sum of exp)
                ot = ot_p.tile([P, 512], BF16, tag="ot", name=f"ot_{sc}_{p}")
                ots.append(ot)
                for h, po in ((0, po_pair[0]), (1, po_pair[1])):
                    rl = rl_p.tile([VW, 512], F32, tag="rl")
                    nc.vector.reciprocal(rl[DK : DK + 1, :], po[DK : DK + 1, :])
                    # partition_broadcast reads physical partition 0 on HW:
                    # hop the row down first
                    rl0 = rl0_p.tile([1, 512], F32, tag="rl0")
                    nc.sync.dma_start(rl0[:], rl[DK : DK + 1, :])
                    rb = rb_p.tile([DK, 512], F32, tag="rb")
                    nc.gpsimd.partition_broadcast(rb[:], rl0[:], channels=DK)
                    if h == 0:
                        nc.vector.tensor_mul(ot[:DK, :], po[:DK, :], rb[:])
                    else:
                        tmp = tmp_p.tile([DK, 512], BF16, tag="tmp")
                        nc.vector.tensor_mul(tmp[:], po[:DK, :], rb[:])
                        nc.sync.dma_start(ot[DK:P, :], tmp[:])

            # Groups are processed in BURSTS of two: 4 scores MMs (all K=64
            # row-group matmuls -- their LDWEIGHTS pull ahead and the head
            # pairs run concurrently, measured ~100ns/MM), then both exps,
            # then the previous burst's 4 AV MMs. Interleaving AV (full-row
            # stationary) between single scores pairs blocks the LDWEIGHTS
            # pull-ahead and costs ~420ns/slot extra (HW microbenched).
            NWO = 4 * NDC
            WO_SLOTS = (2, 5)  # burst indices where a pending W_O unit runs
            pending_av = None
            post_flush = []  # actions to run right after the next AV flush
            wo_queue = []
            all_ots = {}
            po_pairs = {}
            for sc in range(NSC if phases == "ab" else 0):
                all_ots[sc] = []
                for p in range(NPAIR):
                    fl = fillers.get((sc, p), [])
                    po_e = psAV.tile([P, 512], F32, tag="ps", name=f"poe_{sc}_{p}")
                    po_o = psAV.tile([P, 512], F32, tag="ps", name=f"poo_{sc}_{p}")
                    po_pairs[(sc, p)] = (po_e, po_o)
                    for gb, g0 in enumerate(range(0, NT, 2)):
                        pses = []
                        for g in (g0, g0 + 1):
                            pse = psBig.tile([P, 1024], F32, tag="sc",
                                             name=f"pse_{sc}_{p}_{g}")
                            pses.append(pse)
                            emit_scores(sc, p, g, pse)
                        if interleave and sc == 0 and p == 0:
                            emit_v_unit(g0)      # V(t) just before first AV use
                            emit_v_unit(g0 + 1)
                        if gb < len(fl):
                            fl[gb]()
                        if gb in WO_SLOTS and wo_queue:
                            n = NWO // (NPAIR * len(WO_SLOTS))
                            for _ in range(n):
                                if wo_queue:
                                    wo_queue.pop(0)()
                        avs = []
                        for g, pse in zip((g0, g0 + 1), pses):
                            et = et_p.tile([P, 1024], BF16, tag="exp")
                            nc.scalar.activation(
                                et[:], pse[:], mybir.ActivationFunctionType.Exp,
                                scale=float(scale),
                            )
                            avs.append(make_av(sc, p, g, et, (po_e, po_o)))
                        if pending_av is not None:
                            pending_av()
                            for act in post_flush:
                                act()
                            post_flush = []
                        pending_av = lambda avs=avs: [a() for a in avs]
                    # when this pair's last AVs get flushed (next burst),
                    # normalize it and queue the W_O of the previous s-chunk
                    def after(sc=sc, p=p):
                        emit_normalize(sc, p, po_pairs[(sc, p)], all_ots[sc])
                        if p == NPAIR - 1 and sc > 0:
                            for i in range(NWO):
                                wo_queue.append(
                                    lambda i=i, sc=sc: emit_wo_unit(
                                        sc - 1, all_ots[sc - 1], i
                                    )
                                )
                    post_flush.append(after)
            if pending_av is not None:
                pending_av()
                for act in post_flush:
                    act()
            while wo_queue:
                wo_queue.pop(0)()
            if phases == "ab":
                for i in range(NWO):
                    emit_wo_unit(NSC - 1, all_ots[NSC - 1], i)

    nc.compile()
    return nc


_NC_CACHE = {}


def _get_nc(S, D, HL):
    key = (S, D, HL)
    if key not in _NC_CACHE:
        _NC_CACHE[key] = build_nc(S, D, HL)
    return _NC_CACHE[key]


def prep_core_inputs(x_b, wq_l, wk_l, wv_l, wo_l):
    """Per-core input dict from logical per-core slices.

    x_b [S,D]; wq_l/wk_l/wv_l [HL,D,64]; wo_l [HL*64,D]. Weights are
    reshuffled host-side into partition-major bf16 layouts (see build_nc).
    """
    import ml_dtypes

    bf = ml_dtypes.bfloat16
    S, D = x_b.shape
    HL = wq_l.shape[0]
    ND, NPAIR, NT = D // P, HL // 2, S // P

    def qk_prep(w):
        return np.ascontiguousarray(
            w.reshape(NPAIR, 2, ND, P, DK).transpose(0, 3, 2, 1, 4)
            .reshape(NPAIR, P, ND * P)
        ).astype(bf)

    return {
        "xb": np.ascontiguousarray(x_b).astype(bf),
        "wq": qk_prep(wq_l),
        "wk": qk_prep(wk_l),
        "wv": np.ascontiguousarray(
            wv_l.reshape(HL, ND, P, DK).transpose(2, 1, 0, 3)
            .reshape(P, ND * HL * DK)
        ).astype(bf),
        "wo": np.ascontiguousarray(
            wo_l.reshape(NPAIR, P, D).transpose(1, 0, 2).reshape(P, NPAIR * D)
        ).astype(bf),
    }


def make_in_maps(x, W_Q, W_K, W_V, W_O, n_cores=8):
    """Shard full inputs into per-core in_maps (DP over batch x TP over heads)."""
    B = x.shape[0]
    H = W_Q.shape[0]
    n_groups = n_cores // B
    HL = H // n_groups
    in_maps = []
    for c in range(n_cores):
        b, g = c % B, c // B
        hs = slice(g * HL, (g + 1) * HL)
        in_maps.append(prep_core_inputs(
            x[b], W_Q[hs], W_K[hs], W_V[hs],
            W_O[g * HL * DK : (g + 1) * HL * DK],
        ))
    return in_maps


def kernel(x, W_Q, W_K, W_V, W_O):
    from concourse.bass_utils import run_bass_kernel_spmd

    B, S, D = x.shape
    H = W_Q.shape[0]
    n_cores = 8
    HL = H // (n_cores // B)
    nc = _get_nc(S, D, HL)
    in_maps = make_in_maps(x, W_Q, W_K, W_V, W_O, n_cores)
    res = run_bass_kernel_spmd(nc, in_maps, core_ids=list(range(n_cores)))
    y = np.empty((B, S, D), dtype=np.float32)
    for b in range(B):
        y[b] = res.results[b]["y"]
        for g in range(1, n_cores // B):
            y[b] += res.results[g * B + b]["y"]
    return y


# revision 21
# speedup vs baseline: 1.0222x; 1.0222x over previous
"""Multi-headed attention Trainium2 kernel (v4, bf16, interleaved phases).

Problem: B=4, S=2048, D=1024, H=16, dk=dv=64, fp32 in/out.
  q = einsum("bsd,hdk->bhsk", x, W_Q); k,v similar
  scores = q@k.T/8; attn = softmax(scores); out = attn@v
  y = concat_heads(out) @ W_O

Sharding: 8 cores = 4-way data parallel (batch) x 2-way tensor parallel
(head groups of 8). Core c handles batch c%4, heads 8*(c//4)..+8. Each core
returns a partial y for its batch; host sums the two head-group partials.

All matmul operands bf16 (fp32 PSUM accumulate; absmax rel err ~1e-2 <
2e-2). Everything is SBUF-resident: x^T, Q^T/K^T per head pair, V(+ones
column for the softmax denominator), and all weights. Host passes x
pre-cast to bf16; x^T is produced by XBAR dma_start_transpose (no PE/DVE).

The per-iteration stream is a single software-pipelined sequence ordered
for the in-order PE queue: a short prefix (K and Q(chunk 0) of pair 0),
then per (s-chunk, pair) score/exp/AV groups with the remaining phase-A
projection units (K/Q of later pairs, V per t-tile) injected into the PE
slack of early streams, and W_O of s-chunk sc emitted one chunk late,
spread across the next chunk's pairs (its ot inputs are then long since
ready -- no PE stall, no ACT bubble). ACT (exp of all S^2 scores, the
other near-roofline engine besides PE) starts ~25us into the iteration
instead of after the whole projection phase.
"""

import numpy as np

import concourse.bacc as bacc
import concourse.bass as bass
import concourse.mybir as mybir
import concourse.tile as tile

F32 = mybir.dt.float32
BF16 = mybir.dt.bfloat16
P = 128
DK = 64  # per-head dim; also dv
VW = DK + 1  # 65: v columns + ones column
VB = 128  # v_ones block stride: [v(64) | ones | zero pad] -- 256B-aligned
# stationary rows for the AV matmuls (misaligned 130B rows cost ~140ns/MM)


def build_nc(S, D, HL, num_devices=8, n_iters=1, cfg=None):
    """Build the per-core Bass program. S seq len, D model dim, HL local heads."""
    NSC = S // 512  # s-chunks (phase B)
    NT = S // P  # t-tiles
    ND = D // P  # d-tiles
    NPAIR = HL // 2
    NDC = max(1, D // 512)  # output d chunks
    DC = min(D, 512)
    scale = 1.0 / np.sqrt(np.float64(DK))
    cfg = dict(cfg or {})
    phases = cfg.get("phases", "ab")
    interleave = cfg.get("interleave", 1)
    et_bufs = cfg.get("et_bufs", 5)
    ot_bufs = cfg.get("ot_bufs", 12)
    psBig_bufs = cfg.get("psBig_bufs", 2)
    psAV_bufs = cfg.get("psAV_bufs", 4)
    units_big = cfg.get("units_big", 0)

    nc = bacc.Bacc("TRN2", target_bir_lowering=False, debug=False,
                   num_devices=num_devices)

    # host-preshuffled bf16 inputs, partition-major, 2KB+ contiguous rows:
    # xb: x cast to bf16 (host-side)
    # wq/wk [NPAIR, 128, ND*128]: row p = [w(2p)[dt*128+p, :] | w(2p+1)[dt*128+p, :]]_dt
    # wv [128, ND*HL*64]: row p = [wv[hl][dt*128+p, :]]_(dt, hl)
    # wo [128, NPAIR*D]: row p = [wo[pr*128+p, :]]_pr
    xb = nc.dram_tensor("xb", [S, D], BF16, kind="ExternalInput").ap()
    wq = nc.dram_tensor("wq", [NPAIR, P, ND * P], BF16, kind="ExternalInput").ap()
    wk = nc.dram_tensor("wk", [NPAIR, P, ND * P], BF16, kind="ExternalInput").ap()
    wv = nc.dram_tensor("wv", [P, ND * HL * DK], BF16, kind="ExternalInput").ap()
    wo = nc.dram_tensor("wo", [P, NPAIR * D], BF16, kind="ExternalInput").ap()
    y = nc.dram_tensor("y", [S, D], F32, kind="ExternalOutput").ap()

    from contextlib import ExitStack

    with tile.TileContext(nc) as tc:
        with ExitStack() as ctx:
            pool = lambda name, bufs, **kw: ctx.enter_context(
                tc.tile_pool(name=name, bufs=bufs, **kw)
            )
            persist = pool("persist", 1)
            et_p = pool("exp", et_bufs)
            ot_p = pool("ot", ot_bufs)
            y_p = pool("ysb", 3)
            rl_p = pool("rl", 3)
            rb_p = pool("rb", 3)
            tmp_p = pool("tmp", 2)
            rl0_p = pool("rl0", 3)
            psBig = pool("psBig", psBig_bufs, space="PSUM")  # [128,1024] x2 = 4 banks
            psAV = pool("psAV", psAV_bufs, space="PSUM")     # [128,512] x4 = 4 banks

            # --- persistent tiles: loaded/initialized ONCE (outside For_i) ---
            wqk_sb = []
            for pr in range(NPAIR):
                for w_dram, nm in ((wq, "wq"), (wk, "wk")):
                    wt = persist.tile([P, ND * P], BF16, tag=f"{nm}{pr}", name=f"{nm}{pr}")
                    nc.sync.dma_start(wt[:], w_dram[pr])
                    wqk_sb.append(wt)  # index 2*pr (+1 for wk)

            wv_sb = persist.tile([P, ND * HL * DK], BF16, tag="wv")
            nc.sync.dma_start(wv_sb[:], wv[:])
            wos_all = persist.tile([P, NPAIR * D], BF16, tag="wo")
            nc.sync.dma_start(wos_all[:], wo[:])

            xT = [persist.tile([P, S], BF16, tag=f"xT{dt}", name=f"xT{dt}") for dt in range(ND)]
            kt = [persist.tile([P, S], BF16, tag=f"kt{p}", name=f"kt{p}") for p in range(NPAIR)]
            qt = [persist.tile([P, S], BF16, tag=f"qt{p}", name=f"qt{p}") for p in range(NPAIR)]
            v_ones = persist.tile([P, NT * HL * VB], BF16, tag="vones")
            nc.gpsimd.memset(v_ones[:], 0.0)  # zero the pad columns
            ones_view = v_ones[:].rearrange(
                "p (t h c) -> p (t h) c", h=HL, c=VB
            )[:, :, DK : DK + 1]
            nc.gpsimd.memset(ones_view, 1.0)

            if n_iters > 1:
                ctx.enter_context(tc.For_i(0, n_iters, 1))

            # --- x^T via XBAR DMA transpose (no PE/DVE involvement) ---
            xq_eng = nc.scalar if cfg.get("xq", "sp") == "act" else nc.sync
            for dt in range(ND):
                xq_eng.dma_start_transpose(
                    xT[dt][:], xb[:, dt * P : (dt + 1) * P]
                )

            # --- phase-A unit emitters (each: one 8-matmul PSUM group + copy,
            # optionally emitted as two 4-matmul halves to smooth PE load) ---
            def emit_qk_half(pr, wi, sh, half, state={}):
                """Project q (wi=0) or k (wi=1) of pair pr for s-chunk sh."""
                dst = (qt if wi == 0 else kt)[pr]
                wt = wqk_sb[2 * pr + wi]
                if half == 0:
                    if units_big:
                        ps = psBig.tile([P, 1024], F32, tag="sc", name=f"qk_{pr}_{wi}_{sh}")[:, :512]
                    else:
                        ps = psBig.tile([P, 512], F32, tag="sc", name=f"qk_{pr}_{wi}_{sh}")
                    state[(pr, wi, sh)] = ps
                    dts = range(0, ND // 2)
                else:
                    ps = state.pop((pr, wi, sh))
                    dts = range(ND // 2, ND)
                for dt in dts:
                    nc.tensor.matmul(
                        ps[:],
                        wt[:, dt * P : (dt + 1) * P],
                        xT[dt][:, sh * 512 : (sh + 1) * 512],
                        start=(dt == 0),
                        stop=(dt == ND - 1),
                    )
                if half == 1:
                    nc.vector.tensor_copy(dst[:, sh * 512 : (sh + 1) * 512], ps[:])

            def emit_qk_unit(pr, wi, sh):
                emit_qk_half(pr, wi, sh, 0)
                emit_qk_half(pr, wi, sh, 1)

            def emit_v_unit(tt):
                if units_big:
                    ps = psBig.tile([P, 1024], F32, tag="sc", name=f"v_{tt}")[:, :512]
                else:
                    ps = psAV.tile([P, 512], F32, tag="ps", name=f"v_{tt}")
                for dt in range(ND):
                    nc.tensor.matmul(
                        ps[:, : HL * DK],
                        xT[dt][:, tt * P : (tt + 1) * P],
                        wv_sb[:, dt * HL * DK : (dt + 1) * HL * DK],
                        start=(dt == 0),
                        stop=(dt == ND - 1),
                    )
                nc.vector.tensor_copy(
                    v_ones[:].rearrange("p (t h c) -> p t h c", h=HL, c=VB)[
                        :, tt, :, :DK
                    ],
                    ps[:, : HL * DK].rearrange("p (h k) -> p h k", h=HL),
                )

            def emit_wo_unit(sc, ots, i):
                dc, st = i // 4, i % 4
                if units_big:
                    psy = psBig.tile([P, 1024], F32, tag="sc", name=f"psy_{sc}_{i}")[:, :512]
                else:
                    psy = psAV.tile([P, 512], F32, tag="ps", name=f"psy_{sc}_{i}")
                for p in range(NPAIR):
                    nc.tensor.matmul(
                        psy[:, :DC],
                        ots[p][:, st * P : (st + 1) * P],
                        wos_all[:, p * D + dc * DC : p * D + (dc + 1) * DC],
                        start=(p == 0),
                        stop=(p == NPAIR - 1),
                    )
                ys = y_p.tile([P, DC], F32, tag="ysb", name=f"ys_{sc}_{i}")
                nc.vector.tensor_copy(ys[:], psy[:, :DC])
                nc.sync.dma_start(
                    y[(sc * 4 + st) * P : (sc * 4 + st + 1) * P,
                      dc * DC : (dc + 1) * DC],
                    ys[:],
                )

            # --- build the phase-A work schedule ---
            if interleave and phases == "ab":
                # prefix: K(p0) fully + Q(p0, sh0); everything else is
                # injected into per-(sc, p) streams as 4-matmul halves,
                # one per g-slot (smoother PE load than full 8-MM units).
                for sh in range(NSC):
                    emit_qk_unit(0, 1, sh)
                emit_qk_unit(0, 0, 0)

                split = cfg.get("split_fillers", 0)

                def halves(units):
                    out = []
                    for pr, wi, sh in units:
                        if split:
                            out.append(lambda pr=pr, wi=wi, sh=sh: emit_qk_half(pr, wi, sh, 0))
                            out.append(lambda pr=pr, wi=wi, sh=sh: emit_qk_half(pr, wi, sh, 1))
                        else:
                            out.append(lambda pr=pr, wi=wi, sh=sh: emit_qk_unit(pr, wi, sh))
                    return out

                fillers = {}  # (sc, p) -> list of thunks
                for pr in (1, 2, 3)[: NPAIR - 1]:
                    units = [(pr, 1, sh) for sh in range(NSC)] + [(pr, 0, 0)]
                    fillers[(0, pr - 1)] = halves(units)
                # Q(p, sh) for sh>=1: inject during s-chunk sh-1
                for sh in range(1, NSC):
                    if sh == 1:
                        # all four Q(p,1) units go into (sc0, p3)
                        fillers[(0, NPAIR - 1)] = halves(
                            [(pr, 0, 1) for pr in range(NPAIR)]
                        )
                    else:
                        for pr in range(NPAIR):
                            fillers.setdefault((sh - 1, pr), []).extend(
                                halves([(pr, 0, sh)])
                            )
            else:
                # serial phase A (ablation)
                for pr in range(NPAIR):
                    for wi in (0, 1):
                        for sh in range(NSC):
                            emit_qk_unit(pr, wi, sh)
                for tt in range(NT):
                    emit_v_unit(tt)
                fillers = {}

            # --- phase B: flat slot stream with AV lagging one group ---
            # Per slot: scores(g) [+injected phase-A unit], exp(g) on ACT,
            # then AV(g-1). The lag keeps every PE instruction ahead of the
            # exp it feeds: AV(j) waits on exp(j), so putting it AFTER
            # scores/exp of slot j+1 means the in-order PE queue never
            # stalls the ACT feed chain.
            def emit_scores(sc, p, g, pse):
                for h in range(2):  # row-packed head pair
                    nc.tensor.matmul(
                        pse[:, h * 512 : (h + 1) * 512],
                        kt[p][h * DK : (h + 1) * DK, g * P : (g + 1) * P],
                        qt[p][h * DK : (h + 1) * DK, sc * 512 : (sc + 1) * 512],
                        start=True,
                        stop=True,
                    )

            def make_av(sc, p, g, et, po_pair):
                def av():
                    for h, po in ((0, po_pair[0]), (1, po_pair[1])):
                        nc.tensor.matmul(
                            po[:, :],
                            v_ones[
                                :,
                                (g * HL + 2 * p + h) * VB : (g * HL + 2 * p + h + 1) * VB,
                            ],
                            et[:, h * 512 : (h + 1) * 512],
                            start=(g == 0),
                            stop=(g == NT - 1),
                        )
                return av

            def emit_normalize(sc, p, po_pair, ots):
                # normalize: rows 0:64 divided by row 64 (sum of exp)
                ot = ot_p.tile([P, 512], BF16, tag="ot", name=f"ot_{sc}_{p}")
                ots.append(ot)
                for h, po in ((0, po_pair[0]), (1, po_pair[1])):
                    rl = rl_p.tile([VW, 512], F32, tag="rl")
                    nc.vector.reciprocal(rl[DK : DK + 1, :], po[DK : DK + 1, :])
                    # partition_broadcast reads physical partition 0 on HW:
                    # hop the row down first
                    rl0 = rl0_p.tile([1, 512], F32, tag="rl0")
                    nc.sync.dma_start(rl0[:], rl[DK : DK + 1, :])
                    rb = rb_p.tile([DK, 512], F32, tag="rb")
                    nc.gpsimd.partition_broadcast(rb[:], rl0[:], channels=DK)
                    if h == 0:
                        nc.vector.tensor_mul(ot[:DK, :], po[:DK, :], rb[:])
                    else:
                        tmp = tmp_p.tile([DK, 512], BF16, tag="tmp")
                        nc.vector.tensor_mul(tmp[:], po[:DK, :], rb[:])
                        nc.sync.dma_start(ot[DK:P, :], tmp[:])

            # Groups are processed in BURSTS of two: 4 scores MMs (all K=64
            # row-group matmuls -- their LDWEIGHTS pull ahead and the head
            # pairs run concurrently, measured ~100ns/MM), then both exps,
            # then the previous burst's 4 AV MMs. Interleaving AV (full-row
            # stationary) between single scores pairs blocks the LDWEIGHTS
            # pull-ahead and costs ~420ns/slot extra (HW microbenched).
            NWO = 4 * NDC
            WO_SLOTS = (2, 5)  # burst indices where a pending W_O unit runs
            pending_av = None
            post_flush = []  # actions to run right after the next AV flush
            wo_queue = []
            all_ots = {}
            po_pairs = {}
            for sc in range(NSC if phases == "ab" else 0):
                all_ots[sc] = []
                for p in range(NPAIR):
                    fl = fillers.get((sc, p), [])
                    po_e = psAV.tile([P, 512], F32, tag="ps", name=f"poe_{sc}_{p}")
                    po_o = psAV.tile([P, 512], F32, tag="ps", name=f"poo_{sc}_{p}")
                    po_pairs[(sc, p)] = (po_e, po_o)
                    for gb, g0 in enumerate(range(0, NT, 2)):
                        pses = []
                        for g in (g0, g0 + 1):
                            pse = psBig.tile([P, 1024], F32, tag="sc",
                                             name=f"pse_{sc}_{p}_{g}")
                            pses.append(pse)
                            emit_scores(sc, p, g, pse)
                        if interleave and sc == 0 and p == 0:
                            emit_v_unit(g0)      # V(t) just before first AV use
                            emit_v_unit(g0 + 1)
                        if gb < len(fl):
                            fl[gb]()
                        if gb in WO_SLOTS and wo_queue:
                            n = NWO // (NPAIR * len(WO_SLOTS))
                            for _ in range(n):
                                if wo_queue:
                                    wo_queue.pop(0)()
                        avs = []
                        for g, pse in zip((g0, g0 + 1), pses):
                            et = et_p.tile([P, 1024], BF16, tag="exp")
                            nc.scalar.activation(
                                et[:], pse[:], mybir.ActivationFunctionType.Exp,
                                scale=float(scale),
                            )
                            avs.append(make_av(sc, p, g, et, (po_e, po_o)))
                        if pending_av is not None:
                            pending_av()
                            for act in post_flush:
                                act()
                            post_flush = []
                        pending_av = lambda avs=avs: [a() for a in avs]
                    # when this pair's last AVs get flushed (next burst),
                    # normalize it and queue the W_O of the previous s-chunk
                    def after(sc=sc, p=p):
                        emit_normalize(sc, p, po_pairs[(sc, p)], all_ots[sc])
                        if p == NPAIR - 1 and sc > 0:
                            for i in range(NWO):
                                wo_queue.append(
                                    lambda i=i, sc=sc: emit_wo_unit(
                                        sc - 1, all_ots[sc - 1], i
                                    )
                                )
                    post_flush.append(after)
            if pending_av is not None:
                pending_av()
                for act in post_flush:
                    act()
            while wo_queue:
                wo_queue.pop(0)()
            if phases == "ab":
                for i in range(NWO):
                    emit_wo_unit(NSC - 1, all_ots[NSC - 1], i)

    nc.compile()
    return nc


_NC_CACHE = {}


def _get_nc(S, D, HL):
    key = (S, D, HL)
    if key not in _NC_CACHE:
        _NC_CACHE[key] = build_nc(S, D, HL)
    return _NC_CACHE[key]


def prep_core_inputs(x_b, wq_l, wk_l, wv_l, wo_l):
    """Per-core input dict from logical per-core slices.

    x_b [S,D]; wq_l/wk_l/wv_l [HL,D,64]; wo_l [HL*64,D]. Weights are
    reshuffled host-side into partition-major bf16 layouts (see build_nc).
    """
    import ml_dtypes

    bf = ml_dtypes.bfloat16
    S, D = x_b.shape
    HL = wq_l.shape[0]
    ND, NPAIR, NT = D // P, HL // 2, S // P

    def qk_prep(w):
        return np.ascontiguousarray(
            w.reshape(NPAIR, 2, ND, P, DK).transpose(0, 3, 2, 1, 4)
            .reshape(NPAIR, P, ND * P)
        ).astype(bf)

    return {
        "xb": np.ascontiguousarray(x_b).astype(bf),
        "wq": qk_prep(wq_l),
        "wk": qk_prep(wk_l),
        "wv": np.ascontiguousarray(
            wv_l.reshape(HL, ND, P, DK).transpose(2, 1, 0, 3)
            .reshape(P, ND * HL * DK)
        ).astype(bf),
        "wo": np.ascontiguousarray(
            wo_l.reshape(NPAIR, P, D).transpose(1, 0, 2).reshape(P, NPAIR * D)
        ).astype(bf),
    }


def make_in_maps(x, W_Q, W_K, W_V, W_O, n_cores=8):
    """Shard full inputs into per-core in_maps (DP over batch x TP over heads)."""
    B = x.shape[0]
    H = W_Q.shape[0]
    n_groups = n_cores // B
    HL = H // n_groups
    in_maps = []
    for c in range(n_cores):
        b, g = c % B, c // B
        hs = slice(g * HL, (g + 1) * HL)
        in_maps.append(prep_core_inputs(
            x[b], W_Q[hs], W_K[hs], W_V[hs],
            W_O[g * HL * DK : (g + 1) * HL * DK],
        ))
    return in_maps


def kernel(x, W_Q, W_K, W_V, W_O):
    from concourse.bass_utils import run_bass_kernel_spmd

    B, S, D = x.shape
    H = W_Q.shape[0]
    n_cores = 8
    HL = H // (n_cores // B)
    nc = _get_nc(S, D, HL)
    in_maps = make_in_maps(x, W_Q, W_K, W_V, W_O, n_cores)
    res = run_bass_kernel_spmd(nc, in_maps, core_ids=list(range(n_cores)))
    y = np.empty((B, S, D), dtype=np.float32)
    for b in range(B):
        y[b] = res.results[b]["y"]
        for g in range(1, n_cores // B):
            y[b] += res.results[g * B + b]["y"]
    return y


# revision 22
# speedup vs baseline: 1.0369x; 1.0145x over previous
"""Multi-headed attention Trainium2 kernel (v4, bf16, interleaved phases).

Problem: B=4, S=2048, D=1024, H=16, dk=dv=64, fp32 in/out.
  q = einsum("bsd,hdk->bhsk", x, W_Q); k,v similar
  scores = q@k.T/8; attn = softmax(scores); out = attn@v
  y = concat_heads(out) @ W_O

Sharding: 8 cores = 4-way data parallel (batch) x 2-way tensor parallel
(head groups of 8). Core c handles batch c%4, heads 8*(c//4)..+8. Each core
returns a partial y for its batch; host sums the two head-group partials.

All matmul operands bf16 (fp32 PSUM accumulate; absmax rel err ~1e-2 <
2e-2). Everything is SBUF-resident: x^T, Q^T/K^T per head pair, V(+ones
column for the softmax denominator), and all weights. Host passes x
pre-cast to bf16; x^T is produced by XBAR dma_start_transpose (no PE/DVE).

The per-iteration stream is a single software-pipelined sequence ordered
for the in-order PE queue: a short prefix (K and Q(chunk 0) of pair 0),
then per (s-chunk, pair) the t-groups in BURSTS of two -- 4 scores
matmuls (K=64 row-group pairs run concurrently on HW, ~101ns/MM), both
exps, then the previous burst's 4 AV matmuls (full-row stationaries
between single scores pairs would block the LDWEIGHTS pull-ahead,
HW-measured +420ns/slot). AV stationaries are padded to 128-column
256B-aligned blocks (a 65-col 130B-row stationary costs ~+140ns/MM);
pad columns are zero so the extra output rows accumulate zeros. The
remaining phase-A projection units (K/Q of later pairs, V per t-tile)
are injected into the PE slack of early streams, and W_O of s-chunk sc
runs one chunk late, spread across the next chunk's pairs (its ot
inputs are then long since ready -- no PE stall, no ACT bubble). ACT
(exp of all S^2 scores; sustained ~1.1us per [128,1024] instruction on
HW, the hard ~280us/core floor) starts ~25us into the iteration instead
of after the whole projection phase.
"""

import numpy as np

import concourse.bacc as bacc
import concourse.bass as bass
import concourse.mybir as mybir
import concourse.tile as tile

F32 = mybir.dt.float32
BF16 = mybir.dt.bfloat16
P = 128
DK = 64  # per-head dim; also dv
VW = DK + 1  # 65: v columns + ones column
VB = 128  # v_ones block stride: [v(64) | ones | zero pad] -- 256B-aligned
# stationary rows for the AV matmuls (misaligned 130B rows cost ~140ns/MM)


def build_nc(S, D, HL, num_devices=8, n_iters=1, cfg=None):
    """Build the per-core Bass program. S seq len, D model dim, HL local heads."""
    NSC = S // 512  # s-chunks (phase B)
    NT = S // P  # t-tiles
    ND = D // P  # d-tiles
    NPAIR = HL // 2
    NDC = max(1, D // 512)  # output d chunks
    DC = min(D, 512)
    scale = 1.0 / np.sqrt(np.float64(DK))
    cfg = dict(cfg or {})
    phases = cfg.get("phases", "ab")
    interleave = cfg.get("interleave", 1)
    et_bufs = cfg.get("et_bufs", 5)
    ot_bufs = cfg.get("ot_bufs", 12)
    psBig_bufs = cfg.get("psBig_bufs", 2)
    psAV_bufs = cfg.get("psAV_bufs", 4)
    units_big = cfg.get("units_big", 0)

    nc = bacc.Bacc("TRN2", target_bir_lowering=False, debug=False,
                   num_devices=num_devices)

    # host-preshuffled bf16 inputs, partition-major, 2KB+ contiguous rows:
    # xb: x cast to bf16 (host-side)
    # wq/wk [NPAIR, 128, ND*128]: row p = [w(2p)[dt*128+p, :] | w(2p+1)[dt*128+p, :]]_dt
    # wv [128, ND*HL*64]: row p = [wv[hl][dt*128+p, :]]_(dt, hl)
    # wo [128, NPAIR*D]: row p = [wo[pr*128+p, :]]_pr
    xb = nc.dram_tensor("xb", [S, D], BF16, kind="ExternalInput").ap()
    wq = nc.dram_tensor("wq", [NPAIR, P, ND * P], BF16, kind="ExternalInput").ap()
    wk = nc.dram_tensor("wk", [NPAIR, P, ND * P], BF16, kind="ExternalInput").ap()
    wv = nc.dram_tensor("wv", [P, ND * HL * DK], BF16, kind="ExternalInput").ap()
    wo = nc.dram_tensor("wo", [P, NPAIR * D], BF16, kind="ExternalInput").ap()
    y = nc.dram_tensor("y", [S, D], F32, kind="ExternalOutput").ap()

    from contextlib import ExitStack

    with tile.TileContext(nc) as tc:
        with ExitStack() as ctx:
            pool = lambda name, bufs, **kw: ctx.enter_context(
                tc.tile_pool(name=name, bufs=bufs, **kw)
            )
            persist = pool("persist", 1)
            et_p = pool("exp", et_bufs)
            ot_p = pool("ot", ot_bufs)
            y_p = pool("ysb", 3)
            rl_p = pool("rl", 3)
            rb_p = pool("rb", 3)
            tmp_p = pool("tmp", 2)
            rl0_p = pool("rl0", 3)
            psBig = pool("psBig", psBig_bufs, space="PSUM")  # [128,1024] x2 = 4 banks
            psAV = pool("psAV", psAV_bufs, space="PSUM")     # [128,512] x4 = 4 banks

            # --- persistent tiles: loaded/initialized ONCE (outside For_i) ---
            wqk_sb = []
            for pr in range(NPAIR):
                for w_dram, nm in ((wq, "wq"), (wk, "wk")):
                    wt = persist.tile([P, ND * P], BF16, tag=f"{nm}{pr}", name=f"{nm}{pr}")
                    nc.sync.dma_start(wt[:], w_dram[pr])
                    wqk_sb.append(wt)  # index 2*pr (+1 for wk)

            wv_sb = persist.tile([P, ND * HL * DK], BF16, tag="wv")
            nc.sync.dma_start(wv_sb[:], wv[:])
            wos_all = persist.tile([P, NPAIR * D], BF16, tag="wo")
            nc.sync.dma_start(wos_all[:], wo[:])

            xT = [persist.tile([P, S], BF16, tag=f"xT{dt}", name=f"xT{dt}") for dt in range(ND)]
            kt = [persist.tile([P, S], BF16, tag=f"kt{p}", name=f"kt{p}") for p in range(NPAIR)]
            qt = [persist.tile([P, S], BF16, tag=f"qt{p}", name=f"qt{p}") for p in range(NPAIR)]
            v_ones = persist.tile([P, NT * HL * VB], BF16, tag="vones")
            nc.gpsimd.memset(v_ones[:], 0.0)  # zero the pad columns
            ones_view = v_ones[:].rearrange(
                "p (t h c) -> p (t h) c", h=HL, c=VB
            )[:, :, DK : DK + 1]
            nc.gpsimd.memset(ones_view, 1.0)

            if n_iters > 1:
                ctx.enter_context(tc.For_i(0, n_iters, 1))

            # --- x^T via XBAR DMA transpose (no PE/DVE involvement) ---
            xq_eng = nc.scalar if cfg.get("xq", "sp") == "act" else nc.sync
            for dt in range(ND):
                xq_eng.dma_start_transpose(
                    xT[dt][:], xb[:, dt * P : (dt + 1) * P]
                )

            # --- phase-A unit emitters (each: one 8-matmul PSUM group + copy,
            # optionally emitted as two 4-matmul halves to smooth PE load) ---
            def emit_qk_half(pr, wi, sh, half, state={}):
                """Project q (wi=0) or k (wi=1) of pair pr for s-chunk sh."""
                dst = (qt if wi == 0 else kt)[pr]
                wt = wqk_sb[2 * pr + wi]
                if half == 0:
                    if units_big:
                        ps = psBig.tile([P, 1024], F32, tag="sc", name=f"qk_{pr}_{wi}_{sh}")[:, :512]
                    else:
                        ps = psBig.tile([P, 512], F32, tag="sc", name=f"qk_{pr}_{wi}_{sh}")
                    state[(pr, wi, sh)] = ps
                    dts = range(0, ND // 2)
                else:
                    ps = state.pop((pr, wi, sh))
                    dts = range(ND // 2, ND)
                for dt in dts:
                    nc.tensor.matmul(
                        ps[:],
                        wt[:, dt * P : (dt + 1) * P],
                        xT[dt][:, sh * 512 : (sh + 1) * 512],
                        start=(dt == 0),
                        stop=(dt == ND - 1),
                    )
                if half == 1:
                    nc.vector.tensor_copy(dst[:, sh * 512 : (sh + 1) * 512], ps[:])

            def emit_qk_unit(pr, wi, sh):
                emit_qk_half(pr, wi, sh, 0)
                emit_qk_half(pr, wi, sh, 1)

            def emit_v_unit(tt):
                if units_big:
                    ps = psBig.tile([P, 1024], F32, tag="sc", name=f"v_{tt}")[:, :512]
                else:
                    ps = psAV.tile([P, 512], F32, tag="ps", name=f"v_{tt}")
                for dt in range(ND):
                    nc.tensor.matmul(
                        ps[:, : HL * DK],
                        xT[dt][:, tt * P : (tt + 1) * P],
                        wv_sb[:, dt * HL * DK : (dt + 1) * HL * DK],
                        start=(dt == 0),
                        stop=(dt == ND - 1),
                    )
                nc.vector.tensor_copy(
                    v_ones[:].rearrange("p (t h c) -> p t h c", h=HL, c=VB)[
                        :, tt, :, :DK
                    ],
                    ps[:, : HL * DK].rearrange("p (h k) -> p h k", h=HL),
                )

            def emit_wo_unit(sc, ots, i):
                dc, st = i // 4, i % 4
                if units_big:
                    psy = psBig.tile([P, 1024], F32, tag="sc", name=f"psy_{sc}_{i}")[:, :512]
                else:
                    psy = psAV.tile([P, 512], F32, tag="ps", name=f"psy_{sc}_{i}")
                for p in range(NPAIR):
                    nc.tensor.matmul(
                        psy[:, :DC],
                        ots[p][:, st * P : (st + 1) * P],
                        wos_all[:, p * D + dc * DC : p * D + (dc + 1) * DC],
                        start=(p == 0),
                        stop=(p == NPAIR - 1),
                    )
                ys = y_p.tile([P, DC], F32, tag="ysb", name=f"ys_{sc}_{i}")
                nc.vector.tensor_copy(ys[:], psy[:, :DC])
                nc.sync.dma_start(
                    y[(sc * 4 + st) * P : (sc * 4 + st + 1) * P,
                      dc * DC : (dc + 1) * DC],
                    ys[:],
                )

            # --- build the phase-A work schedule ---
            if interleave and phases == "ab":
                # prefix: K(p0) fully + Q(p0, sh0); everything else is
                # injected into per-(sc, p) streams as 4-matmul halves,
                # one per g-slot (smoother PE load than full 8-MM units).
                for sh in range(NSC):
                    emit_qk_unit(0, 1, sh)
                emit_qk_unit(0, 0, 0)

                split = cfg.get("split_fillers", 0)

                def halves(units):
                    out = []
                    for pr, wi, sh in units:
                        if split:
                            out.append(lambda pr=pr, wi=wi, sh=sh: emit_qk_half(pr, wi, sh, 0))
                            out.append(lambda pr=pr, wi=wi, sh=sh: emit_qk_half(pr, wi, sh, 1))
                        else:
                            out.append(lambda pr=pr, wi=wi, sh=sh: emit_qk_unit(pr, wi, sh))
                    return out

                fillers = {}  # (sc, p) -> list of thunks
                for pr in (1, 2, 3)[: NPAIR - 1]:
                    units = [(pr, 1, sh) for sh in range(NSC)] + [(pr, 0, 0)]
                    fillers[(0, pr - 1)] = halves(units)
                # Q(p, sh) for sh>=1: inject during s-chunk sh-1
                for sh in range(1, NSC):
                    if sh == 1:
                        # all four Q(p,1) units go into (sc0, p3)
                        fillers[(0, NPAIR - 1)] = halves(
                            [(pr, 0, 1) for pr in range(NPAIR)]
                        )
                    else:
                        for pr in range(NPAIR):
                            fillers.setdefault((sh - 1, pr), []).extend(
                                halves([(pr, 0, sh)])
                            )
            else:
                # serial phase A (ablation)
                for pr in range(NPAIR):
                    for wi in (0, 1):
                        for sh in range(NSC):
                            emit_qk_unit(pr, wi, sh)
                for tt in range(NT):
                    emit_v_unit(tt)
                fillers = {}

            # --- phase B: flat slot stream with AV lagging one group ---
            # Per slot: scores(g) [+injected phase-A unit], exp(g) on ACT,
            # then AV(g-1). The lag keeps every PE instruction ahead of the
            # exp it feeds: AV(j) waits on exp(j), so putting it AFTER
            # scores/exp of slot j+1 means the in-order PE queue never
            # stalls the ACT feed chain.
            def emit_scores(sc, p, g, pse):
                for h in range(2):  # row-packed head pair
                    nc.tensor.matmul(
                        pse[:, h * 512 : (h + 1) * 512],
                        kt[p][h * DK : (h + 1) * DK, g * P : (g + 1) * P],
                        qt[p][h * DK : (h + 1) * DK, sc * 512 : (sc + 1) * 512],
                        start=True,
                        stop=True,
                    )

            def make_av(sc, p, g, et, po_pair):
                def av():
                    for h, po in ((0, po_pair[0]), (1, po_pair[1])):
                        nc.tensor.matmul(
                            po[:, :],
                            v_ones[
                                :,
                                (g * HL + 2 * p + h) * VB : (g * HL + 2 * p + h + 1) * VB,
                            ],
                            et[:, h * 512 : (h + 1) * 512],
                            start=(g == 0),
                            stop=(g == NT - 1),
                        )
                return av

            def emit_normalize(sc, p, po_pair, ots):
                # normalize: rows 0:64 divided by row 64 (sum of exp)
                ot = ot_p.tile([P, 512], BF16, tag="ot", name=f"ot_{sc}_{p}")
                ots.append(ot)
                for h, po in ((0, po_pair[0]), (1, po_pair[1])):
                    rl = rl_p.tile([VW, 512], F32, tag="rl")
                    nc.vector.reciprocal(rl[DK : DK + 1, :], po[DK : DK + 1, :])
                    # partition_broadcast reads physical partition 0 on HW:
                    # hop the row down first
                    rl0 = rl0_p.tile([1, 512], F32, tag="rl0")
                    nc.sync.dma_start(rl0[:], rl[DK : DK + 1, :])
                    rb = rb_p.tile([DK, 512], F32, tag="rb")
                    nc.gpsimd.partition_broadcast(rb[:], rl0[:], channels=DK)
                    if h == 0:
                        nc.vector.tensor_mul(ot[:DK, :], po[:DK, :], rb[:])
                    else:
                        tmp = tmp_p.tile([DK, 512], BF16, tag="tmp")
                        nc.vector.tensor_mul(tmp[:], po[:DK, :], rb[:])
                        nc.sync.dma_start(ot[DK:P, :], tmp[:])

            # Groups are processed in BURSTS of two: 4 scores MMs (all K=64
            # row-group matmuls -- their LDWEIGHTS pull ahead and the head
            # pairs run concurrently, measured ~100ns/MM), then both exps,
            # then the previous burst's 4 AV MMs. Interleaving AV (full-row
            # stationary) between single scores pairs blocks the LDWEIGHTS
            # pull-ahead and costs ~420ns/slot extra (HW microbenched).
            NWO = 4 * NDC
            WO_SLOTS = (2, 5)  # burst indices where a pending W_O unit runs
            pending_av = None
            post_flush = []  # actions to run right after the next AV flush
            wo_queue = []
            all_ots = {}
            po_pairs = {}
            for sc in range(NSC if phases == "ab" else 0):
                all_ots[sc] = []
                for p in range(NPAIR):
                    fl = fillers.get((sc, p), [])
                    po_e = psAV.tile([P, 512], F32, tag="ps", name=f"poe_{sc}_{p}")
                    po_o = psAV.tile([P, 512], F32, tag="ps", name=f"poo_{sc}_{p}")
                    po_pairs[(sc, p)] = (po_e, po_o)
                    for gb, g0 in enumerate(range(0, NT, 2)):
                        pses = []
                        for g in (g0, g0 + 1):
                            pse = psBig.tile([P, 1024], F32, tag="sc",
                                             name=f"pse_{sc}_{p}_{g}")
                            pses.append(pse)
                            emit_scores(sc, p, g, pse)
                        if interleave and sc == 0 and p == 0:
                            emit_v_unit(g0)      # V(t) just before first AV use
                            emit_v_unit(g0 + 1)
                        if gb < len(fl):
                            fl[gb]()
                        if gb in WO_SLOTS and wo_queue:
                            n = NWO // (NPAIR * len(WO_SLOTS))
                            for _ in range(n):
                                if wo_queue:
                                    wo_queue.pop(0)()
                        avs = []
                        for g, pse in zip((g0, g0 + 1), pses):
                            et = et_p.tile([P, 1024], BF16, tag="exp")
                            nc.scalar.activation(
                                et[:], pse[:], mybir.ActivationFunctionType.Exp,
                                scale=float(scale),
                            )
                            avs.append(make_av(sc, p, g, et, (po_e, po_o)))
                        if pending_av is not None:
                            pending_av()
                            for act in post_flush:
                                act()
                            post_flush = []
                        pending_av = lambda avs=avs: [a() for a in avs]
                    # when this pair's last AVs get flushed (next burst),
                    # normalize it and queue the W_O of the previous s-chunk
                    def after(sc=sc, p=p):
                        emit_normalize(sc, p, po_pairs[(sc, p)], all_ots[sc])
                        if p == NPAIR - 1 and sc > 0:
                            for i in range(NWO):
                                wo_queue.append(
                                    lambda i=i, sc=sc: emit_wo_unit(
                                        sc - 1, all_ots[sc - 1], i
                                    )
                                )
                    post_flush.append(after)
            if pending_av is not None:
                pending_av()
                for act in post_flush:
                    act()
            while wo_queue:
                wo_queue.pop(0)()
            if phases == "ab":
                for i in range(NWO):
                    emit_wo_unit(NSC - 1, all_ots[NSC - 1], i)

    nc.compile()
    return nc


_NC_CACHE = {}


def _get_nc(S, D, HL):
    key = (S, D, HL)
    if key not in _NC_CACHE:
        _NC_CACHE[key] = build_nc(S, D, HL)
    return _NC_CACHE[key]


def prep_core_inputs(x_b, wq_l, wk_l, wv_l, wo_l):
    """Per-core input dict from logical per-core slices.

    x_b [S,D]; wq_l/wk_l/wv_l [HL,D,64]; wo_l [HL*64,D]. Weights are
    reshuffled host-side into partition-major bf16 layouts (see build_nc).
    """
    import ml_dtypes

    bf = ml_dtypes.bfloat16
    S, D = x_b.shape
    HL = wq_l.shape[0]
    ND, NPAIR, NT = D // P, HL // 2, S // P

    def qk_prep(w):
        return np.ascontiguousarray(
            w.reshape(NPAIR, 2, ND, P, DK).transpose(0, 3, 2, 1, 4)
            .reshape(NPAIR, P, ND * P)
        ).astype(bf)

    return {
        "xb": np.ascontiguousarray(x_b).astype(bf),
        "wq": qk_prep(wq_l),
        "wk": qk_prep(wk_l),
        "wv": np.ascontiguousarray(
            wv_l.reshape(HL, ND, P, DK).transpose(2, 1, 0, 3)
            .reshape(P, ND * HL * DK)
        ).astype(bf),
        "wo": np.ascontiguousarray(
            wo_l.reshape(NPAIR, P, D).transpose(1, 0, 2).reshape(P, NPAIR * D)
        ).astype(bf),
    }


def make_in_maps(x, W_Q, W_K, W_V, W_O, n_cores=8):
    """Shard full inputs into per-core in_maps (DP over batch x TP over heads)."""
    B = x.shape[0]
    H = W_Q.shape[0]
    n_groups = n_cores // B
    HL = H // n_groups
    in_maps = []
    for c in range(n_cores):
        b, g = c % B, c // B
        hs = slice(g * HL, (g + 1) * HL)
        in_maps.append(prep_core_inputs(
            x[b], W_Q[hs], W_K[hs], W_V[hs],
            W_O[g * HL * DK : (g + 1) * HL * DK],
        ))
    return in_maps


def kernel(x, W_Q, W_K, W_V, W_O):
    from concourse.bass_utils import run_bass_kernel_spmd

    B, S, D = x.shape
    H = W_Q.shape[0]
    n_cores = 8
    HL = H // (n_cores // B)
    nc = _get_nc(S, D, HL)
    in_maps = make_in_maps(x, W_Q, W_K, W_V, W_O, n_cores)
    res = run_bass_kernel_spmd(nc, in_maps, core_ids=list(range(n_cores)))
    y = np.empty((B, S, D), dtype=np.float32)
    for b in range(B):
        y[b] = res.results[b]["y"]
        for g in range(1, n_cores // B):
            y[b] += res.results[g * B + b]["y"]
    return y
